# revision 5
# baseline (speedup 1.0000x reference)
"""Causal multi-head attention block (B=16, S=1024, E=256, H=4, D=64) on 8
Trainium2 NeuronCores, data-parallel over batch (2 batches per core).

Layout strategy (per core, per batch):
  - x is PE-transposed to xT [E, S] so QKV projections contract E on the
    partition dim.
  - Q,K are produced TRANSPOSED (qkT [d, S]); V is produced in natural [S, d]
    layout with a ones column appended per head (V_ext) so the P@V matmul
    also yields the softmax denominators (row 64 of the accumulator).
  - Scores are computed as S^T = K Q^T [k, q]; softmax needs no max-subtract
    (scores ~ N(0,1)), so P^T = exp(S^T/8) with causal masking as a 0/1
    multiply on the 4 diagonal block types.
  - Õ^T = V_ext^T P^T accumulates over k-chunks in PSUM; normalization is
    psum * broadcast(1/denom), broadcast done with a K=1 PE matmul.
  - Õ^T is exactly the lhsT the output projection wants.
  - The V-bias is folded through attention algebra into a single combined
    output bias  bc = b_v @ W_out + b_out,  computed once on device.

All matmul operands use float32r (~13-bit mantissa, 4x faster than f32 on
the PE); accumulation stays f32 in PSUM.
"""
import sys

for p in ("/opt/trn_rl_repo",):
    if p not in sys.path:
        sys.path.insert(0, p)

import numpy as np

import concourse.bass as bass
import concourse.mybir as mybir
import concourse.tile as tile
from concourse.masks import make_identity
from concourse.vector_clock import ScopedClock

F32 = mybir.dt.float32
F32R = mybir.dt.float32r
AF = mybir.ActivationFunctionType

N_CORES = 8
B, S, E, H, D = 16, 1024, 256, 4, 64
BPC = B // N_CORES          # batches per core
SC = S // 128               # 128-row s-chunks
QN = S // 512               # 512-col q blocks
WAIT_CAP = 1


class TC(tile.TileContext):
    """TileContext workaround: this walrus build accepts at most one sync
    wait per instruction, so excess waits are peeled onto same-engine NOPs
    emitted immediately before the owning instruction (same semantics: the
    engine blocks on each in order)."""

    def _split_excess_waits(self, inst):
        si = inst.sync_info
        if si is None or len(si.on_wait) <= WAIT_CAP:
            return []
        waits = list(si.on_wait)
        kept, extra = waits[-WAIT_CAP:], waits[:-WAIT_CAP]
        nops = []
        for w in extra:
            nops.append(
                mybir.InstNoOp(
                    name=self.nc.get_next_instruction_name(),
                    engine=inst.engine,
                    sync_info=mybir.SyncInfo(on_wait=[w], on_update=[]),
                    ins=[],
                    outs=[],
                    bass_nofuse=True,
                )
            )
        inst.sync_info = mybir.SyncInfo(on_wait=kept, on_update=list(si.on_update))
        return nops

    def _add_instruction(self, inst):
        for n in self._split_excess_waits(inst):
            super()._add_instruction(n)
        super()._add_instruction(inst)

    def _drain_and_barrier(self, tick_clock, wait_clock):
        probe = self.nc.sync.nop(nofuse=True)
        wait_clock.add_sem_waits(probe.ins, ScopedClock({None: tick_clock.global_clock}))
        si = probe.ins.sync_info
        waits = list(si.on_wait) if si is not None else []
        probe.ins.sync_info = mybir.SyncInfo(on_wait=waits[:1], on_update=[])
        for w in waits[1:]:
            n2 = self.nc.sync.nop(nofuse=True)
            n2.ins.sync_info = mybir.SyncInfo(on_wait=[w], on_update=[])
        self.nc.sync.drain()
        self.nc.all_engine_barrier()
        assert self.sems is not None
        popped = self.nc._tile_sem_poison_stack.pop()
        assert popped is self._sem_poison
        self.nc.clear_and_free_semaphores(list(self.sems.allocated().values()))
        self.nc.all_engine_barrier()


def build_nc():
    nc = bass.Bass()
    x = nc.dram_tensor("x", [BPC, S, E], F32, kind="ExternalInput")
    w_qkv = nc.dram_tensor("W_qkv", [E, 3 * E], F32, kind="ExternalInput")
    b_qkv = nc.dram_tensor("b_qkv", [3 * E], F32, kind="ExternalInput")
    w_out = nc.dram_tensor("W_out", [E, E], F32, kind="ExternalInput")
    b_out = nc.dram_tensor("b_out", [E], F32, kind="ExternalInput")
    out = nc.dram_tensor("out", [BPC, S, E], F32, kind="ExternalOutput")

    with TC(nc) as tc:
        _emit(nc, tc, x, w_qkv, b_qkv, w_out, b_out, out)
    return nc


def _emit(nc, tc, x, w_qkv, b_qkv, w_out, b_out, out):
    import contextlib

    ctx = contextlib.ExitStack()
    with ctx:
        singles = ctx.enter_context(tc.tile_pool(name="singles", bufs=1))
        sb = ctx.enter_context(tc.tile_pool(name="sb", bufs=2))
        ps = ctx.enter_context(tc.tile_pool(name="ps", bufs=2, space="PSUM"))

        # ---------------- one-time setup ----------------
        ident = singles.tile([128, 128], F32, tag="ident")
        make_identity(nc, ident)

        # raw weight loads
        wraw = singles.tile([128, 2, 768], F32, tag="wraw")
        nc.gpsimd.dma_start(out=wraw[:, 0, :], in_=w_qkv.ap()[0:128, :])
        nc.gpsimd.dma_start(out=wraw[:, 1, :], in_=w_qkv.ap()[128:256, :])
        woraw = singles.tile([128, 2, 256], F32, tag="woraw")
        nc.gpsimd.dma_start(out=woraw[:, 0, :], in_=w_out.ap()[0:128, :])
        nc.gpsimd.dma_start(out=woraw[:, 1, :], in_=w_out.ap()[128:256, :])
        # rounded (f32r) copies for the PE
        wr = singles.tile([128, 2, 768], F32R, tag="wr")
        nc.vector.tensor_copy(wr, wraw)
        wor = singles.tile([128, 2, 256], F32R, tag="wor")
        nc.vector.tensor_copy(wor, woraw)

        # b_qkv columns: [128, 4] for the 512 q,k biases (per-partition use),
        # [128, 2] (f32r) for the 256 v biases (matmul lhsT use)
        bqk_col = singles.tile([128, 4], F32, tag="bqk_col")
        nc.gpsimd.dma_start(
            out=bqk_col, in_=b_qkv.ap()[0:512].rearrange("(c p) -> p c", p=128)
        )
        bv_raw = singles.tile([128, 2], F32, tag="bv_raw")
        nc.gpsimd.dma_start(
            out=bv_raw, in_=b_qkv.ap()[512:768].rearrange("(c p) -> p c", p=128)
        )
        bv_col = singles.tile([128, 2], F32R, tag="bv_col")
        nc.vector.tensor_copy(bv_col, bv_raw)

        bout_row = singles.tile([1, 256], F32, tag="bout_row")
        nc.gpsimd.dma_start(out=bout_row, in_=b_out.ap()[None, :])

        # ones vectors (f32r via rounding copy; memset bits would likely be
        # fine but the BIR verifier wants f32r-producing ops)
        ones_f32 = singles.tile([1, 128], F32, tag="ones_f32")
        nc.vector.memset(ones_f32, 1.0)
        ones_r = singles.tile([1, 128], F32R, tag="ones_r")
        nc.vector.tensor_copy(ones_r, ones_f32)

        ones_col4 = singles.tile([128, 4], F32, tag="ones_col4")
        nc.vector.memset(ones_col4, 1.0)
        ones_col4r = singles.tile([128, 4], F32R, tag="ones_col4r")
        nc.vector.tensor_copy(ones_col4r, ones_col4)

        # causal 0/1 masks for the 4 diagonal-block offsets:
        # mask[j][p, y] = 1.0 if (y - p - 128*j) >= 0 else 0.0
        masks = []
        for j in range(4):
            mf = singles.tile([128, 512], F32, tag=f"maskf{j}")
            nc.gpsimd.memset(mf, 1.0)
            nc.gpsimd.affine_select(
                out=mf,
                in_=mf,
                compare_op=mybir.AluOpType.is_ge,
                fill=0.0,
                base=-128 * j,
                pattern=[[1, 512]],
                channel_multiplier=-1,
            )
            mr = singles.tile([128, 512], F32R, tag=f"maskr{j}")
            nc.vector.tensor_copy(mr, mf)
            masks.append(mr)

        # combined output bias bc = b_v @ W_out + b_out, broadcast to [128, 256]
        ps_bv = ps.tile([1, 256], F32, tag="ps256")
        for j in range(2):
            nc.tensor.matmul(
                ps_bv, bv_col[:, j : j + 1], wor[:, j, :], start=(j == 0), stop=(j == 1)
            )
        bc_row = singles.tile([1, 256], F32R, tag="bc_row")
        nc.vector.tensor_add(bc_row, ps_bv, bout_row)
        ps_bc = ps.tile([128, 256], F32, tag="ps256")
        nc.tensor.matmul(ps_bc, ones_r, bc_row, start=True, stop=True)
        bc_sb = singles.tile([128, 256], F32, tag="bc_sb")
        nc.scalar.activation(out=bc_sb, in_=ps_bc, func=AF.Identity)

        # ---------------- per-batch pipeline ----------------
        for b in range(BPC):
            # ---- stage A: load x, build xT [2][128, 1024] (f32r) ----
            xin = []
            for sc in range(SC):
                t = sb.tile([128, 256], F32, tag=f"xin{sc}")
                nc.gpsimd.dma_start(out=t, in_=x.ap()[b, sc * 128 : (sc + 1) * 128, :])
                xin.append(t)
            xT = [sb.tile([128, 1024], F32R, tag=f"xT{ec}", name=f"xT{ec}") for ec in range(2)]
            for ec in range(2):
                for g in range(2):
                    ps_t = ps.tile([128, 512], F32, tag="ps512")
                    for i in range(4):
                        nc.tensor.transpose(
                            ps_t[:, i * 128 : (i + 1) * 128],
                            xin[g * 4 + i][:, ec * 128 : (ec + 1) * 128],
                            ident,
                        )
                    nc.scalar.activation(
                        out=xT[ec][:, g * 512 : (g + 1) * 512], in_=ps_t, func=AF.Identity
                    )

            # ---- stage B: qkT [4][128, 1024] = (W_qk)^T xT + b (f32r) ----
            qkT = [sb.tile([128, 1024], F32R, tag=f"qkT{m}", name=f"qkT{m}") for m in range(4)]
            for m in range(4):
                for qb in range(QN):
                    ps_qk = ps.tile([128, 512], F32, tag="ps512")
                    for ec in range(2):
                        nc.tensor.matmul(
                            ps_qk,
                            wr[:, ec, m * 128 : (m + 1) * 128],
                            xT[ec][:, qb * 512 : (qb + 1) * 512],
                            start=(ec == 0),
                            stop=(ec == 1),
                        )
                    nc.scalar.activation(
                        out=qkT[m][:, qb * 512 : (qb + 1) * 512],
                        in_=ps_qk,
                        func=AF.Identity,
                        bias=bqk_col[:, m : m + 1],
                    )

            # ---- stage B2: V_ext [8][128, 260] natural V + ones cols ----
            vext = []
            for sc in range(SC):
                ps_v = ps.tile([128, 256], F32, tag="ps256")
                for ec in range(2):
                    nc.tensor.matmul(
                        ps_v,
                        xT[ec][:, sc * 128 : (sc + 1) * 128],
                        wr[:, ec, 512:768],
                        start=(ec == 0),
                        stop=(ec == 1),
                    )
                v = sb.tile([128, 4, 65], F32R, tag=f"vext{sc}")
                nc.vector.tensor_copy(
                    v[:, :, 0:64], ps_v.rearrange("p (h d) -> p h d", d=64)
                )
                nc.vector.tensor_copy(v[:, :, 64:65], ones_col4r[:, :, None])
                vext.append(v)

            # ---- stage C: attention per head ----
            ocatT = [sb.tile([128, 1024], F32R, tag=f"ocatT{hc}", name=f"ocatT{hc}") for hc in range(2)]
            for h in range(H):
                hc, hr = h // 2, (h % 2) * 64
                qT = qkT[hc]
                kT = qkT[2 + hc]
                for qb in range(QN):
                    ps_o = ps.tile([65, 512], F32, tag="pso")
                    kmax = 4 * qb + 3
                    for kc in range(kmax + 1):
                        ps_s = ps.tile([128, 512], F32, tag="ps512")
                        nc.tensor.matmul(
                            ps_s,
                            kT[hr : hr + 64, kc * 128 : (kc + 1) * 128],
                            qT[hr : hr + 64, qb * 512 : (qb + 1) * 512],
                            start=True,
                            stop=True,
                        )
                        pT = sb.tile([128, 512], F32R, tag="pT")
                        nc.scalar.activation(out=pT, in_=ps_s, func=AF.Exp, scale=0.125)
                        j = kc - 4 * qb
                        if j >= 0:
                            nc.vector.tensor_mul(pT, pT, masks[j])
                        nc.tensor.matmul(
                            ps_o,
                            vext[kc][:, h, :],
                            pT,
                            start=(kc == 0),
                            stop=(kc == kmax),
                        )
                    # normalize: ocatT rows [hr:hr+64] = psum_o * bcast(1/denom)
                    recip = sb.tile([1, 512], F32R, tag="recip")
                    with nc.allow_low_precision(reason="f32r rounding of softmax denom reciprocals"):
                        nc.vector.reciprocal(recip, ps_o[64:65, :])
                    ps_n = ps.tile([64, 512], F32, tag="psn")
                    nc.tensor.matmul(ps_n, ones_r[:, 0:64], recip, start=True, stop=True)
                    sb_n = sb.tile([64, 512], F32, tag="sbn")
                    nc.vector.tensor_copy(sb_n, ps_n)
                    nc.vector.tensor_mul(
                        ocatT[hc][hr : hr + 64, qb * 512 : (qb + 1) * 512],
                        ps_o[0:64, :],
                        sb_n,
                    )

            # ---- stage D: output projection + combined bias ----
            for sc in range(SC):
                ps_out = ps.tile([128, 256], F32, tag="ps256")
                for hc in range(2):
                    nc.tensor.matmul(
                        ps_out,
                        ocatT[hc][:, sc * 128 : (sc + 1) * 128],
                        wor[:, hc, :],
                        start=(hc == 0),
                        stop=(hc == 1),
                    )
                o = sb.tile([128, 256], F32, tag="osb")
                nc.vector.tensor_add(o, ps_out, bc_sb)
                nc.gpsimd.dma_start(
                    out=out.ap()[b, sc * 128 : (sc + 1) * 128, :], in_=o
                )


# ---------------- host-side runner ----------------
_RUNNER = None


class _Runner:
    """Compile once, run many: replicates bass2jax.run_bass_via_pjrt's
    shard_map-over-8-devices path but caches the jitted callable."""

    def __init__(self):
        import jax
        from jax.sharding import Mesh, PartitionSpec
        from jax.experimental.shard_map import shard_map
        from concourse import bass2jax

        bass2jax.install_neuronx_cc_hook()
        nc = build_nc()
        self.nc = nc

        partition_name = nc.partition_id_tensor.name if nc.partition_id_tensor else None
        in_names, out_names, out_avals, zero_outs = [], [], [], []
        for alloc in nc.m.functions[0].allocations:
            if not isinstance(alloc, mybir.MemoryLocationSet):
                continue
            name = alloc.memorylocations[0].name
            if alloc.kind == "ExternalInput":
                if name != partition_name:
                    in_names.append(name)
            elif alloc.kind == "ExternalOutput":
                out_names.append(name)
                shape = tuple(alloc.tensor_shape)
                dtype = mybir.dt.np(alloc.dtype)
                out_avals.append(jax.core.ShapedArray(shape, dtype))
                zero_outs.append(np.zeros(shape, dtype))
        self.n_params = len(in_names)
        n_outs = len(out_avals)
        self.in_names = list(in_names)
        self.out_names = out_names
        self.out_avals = out_avals
        self.zero_outs = zero_outs
        in_names = in_names + out_names
        if partition_name is not None:
            in_names.append(partition_name)

        def _body(*args):
            operands = list(args)
            if partition_name is not None:
                operands.append(bass2jax.partition_id_tensor())
            outs = bass2jax._bass_exec_p.bind(
                *operands,
                out_avals=tuple(out_avals),
                in_names=tuple(in_names),
                out_names=tuple(out_names),
                lowering_input_output_aliases=(),
                sim_require_finite=True,
                sim_require_nnan=True,
                nc=nc,
            )
            return tuple(outs)

        devices = jax.devices()[:N_CORES]
        mesh = Mesh(np.asarray(devices), ("core",))
        in_specs = (PartitionSpec("core"),) * (self.n_params + n_outs)
        out_specs = (PartitionSpec("core"),) * len(out_names)
        self.fn = jax.jit(
            shard_map(
                _body, mesh=mesh, in_specs=in_specs, out_specs=out_specs, check_rep=False
            ),
            donate_argnums=tuple(range(self.n_params, self.n_params + n_outs)),
            keep_unused=True,
        )

    def concat_inputs(self, in_maps):
        return [
            np.concatenate([np.asarray(m[name]) for m in in_maps], axis=0)
            for name in self.in_names
        ]

    def run_async(self, concat_in):
        concat_zeros = [
            np.zeros((N_CORES * z.shape[0], *z.shape[1:]), z.dtype)
            for z in self.zero_outs
        ]
        return self.fn(*concat_in, *concat_zeros)

    def run(self, in_maps):
        out_arrs = self.run_async(self.concat_inputs(in_maps))
        return [
            {
                name: np.asarray(out_arrs[i]).reshape(
                    N_CORES, *self.out_avals[i].shape
                )[c]
                for i, name in enumerate(self.out_names)
            }
            for c in range(N_CORES)
        ]


def _get_runner():
    global _RUNNER
    if _RUNNER is None:
        _RUNNER = _Runner()
    return _RUNNER


def kernel(x, W_qkv, b_qkv, W_out, b_out):
    x = np.ascontiguousarray(np.asarray(x, dtype=np.float32))
    in_maps = [
        {
            "x": x[c * BPC : (c + 1) * BPC],
            "W_qkv": np.asarray(W_qkv, np.float32),
            "b_qkv": np.asarray(b_qkv, np.float32),
            "W_out": np.asarray(W_out, np.float32),
            "b_out": np.asarray(b_out, np.float32),
        }
        for c in range(N_CORES)
    ]
    results = _get_runner().run(in_maps)
    return np.concatenate([r["out"] for r in results], axis=0)


# revision 6
# speedup vs baseline: 69.6811x; 69.6811x over previous
"""Causal multi-head attention block (B=16, S=1024, E=256, H=4, D=64) on 8
Trainium2 NeuronCores, data-parallel over batch (2 batches per core).

Layout strategy (per core, per batch):
  - x is PE-transposed to xT [E, S] so QKV projections contract E on the
    partition dim.
  - Q,K are produced TRANSPOSED (qkT [d, S]); V is produced in natural [S, d]
    layout with a ones column appended per head (V_ext) so the P@V matmul
    also yields the softmax denominators (row 64 of the accumulator).
  - Scores are computed as S^T = K Q^T [k, q]; softmax needs no max-subtract
    (scores ~ N(0,1)), so P^T = exp(S^T/8) with causal masking as a 0/1
    multiply on the 4 diagonal block types.
  - Õ^T = V_ext^T P^T accumulates over k-chunks in PSUM; normalization is
    psum * broadcast(1/denom), broadcast done with a K=1 PE matmul.
  - Õ^T is exactly the lhsT the output projection wants.
  - The V-bias is folded through attention algebra into a single combined
    output bias  bc = b_v @ W_out + b_out,  computed once on device.

All matmul operands use float32r (~13-bit mantissa, 4x faster than f32 on
the PE); accumulation stays f32 in PSUM.
"""
import sys

for p in ("/opt/trn_rl_repo",):
    if p not in sys.path:
        sys.path.insert(0, p)

import numpy as np

import concourse.bass as bass
import concourse.mybir as mybir
import concourse.tile as tile
from concourse.masks import make_identity
from concourse.vector_clock import ScopedClock

F32 = mybir.dt.float32
F32R = mybir.dt.float32r
AF = mybir.ActivationFunctionType

N_CORES = 8
B, S, E, H, D = 16, 1024, 256, 4, 64
BPC = B // N_CORES          # batches per core
SC = S // 128               # 128-row s-chunks
QN = S // 512               # 512-col q blocks
WAIT_CAP = 1


class TC(tile.TileContext):
    """TileContext workaround: this walrus build accepts at most one sync
    wait per instruction, so excess waits are peeled onto same-engine NOPs
    emitted immediately before the owning instruction (same semantics: the
    engine blocks on each in order)."""

    def _split_excess_waits(self, inst):
        si = inst.sync_info
        if si is None or len(si.on_wait) <= WAIT_CAP:
            return []
        waits = list(si.on_wait)
        kept, extra = waits[-WAIT_CAP:], waits[:-WAIT_CAP]
        nops = []
        for w in extra:
            nops.append(
                mybir.InstNoOp(
                    name=self.nc.get_next_instruction_name(),
                    engine=inst.engine,
                    sync_info=mybir.SyncInfo(on_wait=[w], on_update=[]),
                    ins=[],
                    outs=[],
                    bass_nofuse=True,
                )
            )
        inst.sync_info = mybir.SyncInfo(on_wait=kept, on_update=list(si.on_update))
        return nops

    def _add_instruction(self, inst):
        for n in self._split_excess_waits(inst):
            super()._add_instruction(n)
        super()._add_instruction(inst)

    def _drain_and_barrier(self, tick_clock, wait_clock):
        probe = self.nc.sync.nop(nofuse=True)
        wait_clock.add_sem_waits(probe.ins, ScopedClock({None: tick_clock.global_clock}))
        si = probe.ins.sync_info
        waits = list(si.on_wait) if si is not None else []
        probe.ins.sync_info = mybir.SyncInfo(on_wait=waits[:1], on_update=[])
        for w in waits[1:]:
            n2 = self.nc.sync.nop(nofuse=True)
            n2.ins.sync_info = mybir.SyncInfo(on_wait=[w], on_update=[])
        self.nc.sync.drain()
        self.nc.all_engine_barrier()
        assert self.sems is not None
        popped = self.nc._tile_sem_poison_stack.pop()
        assert popped is self._sem_poison
        self.nc.clear_and_free_semaphores(list(self.sems.allocated().values()))
        self.nc.all_engine_barrier()


def build_nc(reps: int = 1):
    nc = bass.Bass()
    x = nc.dram_tensor("x", [BPC, S, E], F32, kind="ExternalInput")
    w_qkv = nc.dram_tensor("W_qkv", [E, 3 * E], F32, kind="ExternalInput")
    b_qkv = nc.dram_tensor("b_qkv", [3 * E], F32, kind="ExternalInput")
    w_out = nc.dram_tensor("W_out", [E, E], F32, kind="ExternalInput")
    b_out = nc.dram_tensor("b_out", [E], F32, kind="ExternalInput")
    out = nc.dram_tensor("out", [BPC, S, E], F32, kind="ExternalOutput")

    with TC(nc) as tc:
        _emit(nc, tc, x, w_qkv, b_qkv, w_out, b_out, out, reps)
    return nc


def _emit(nc, tc, x, w_qkv, b_qkv, w_out, b_out, out, reps=1):
    import contextlib

    ctx = contextlib.ExitStack()
    with ctx:
        singles = ctx.enter_context(tc.tile_pool(name="singles", bufs=1))
        sb = ctx.enter_context(tc.tile_pool(name="sb", bufs=2))
        ps = ctx.enter_context(tc.tile_pool(name="ps", bufs=2, space="PSUM"))

        # ---------------- one-time setup ----------------
        ident = singles.tile([128, 128], F32, tag="ident")
        make_identity(nc, ident)

        # raw weight loads
        wraw = singles.tile([128, 2, 768], F32, tag="wraw")
        nc.gpsimd.dma_start(out=wraw[:, 0, :], in_=w_qkv.ap()[0:128, :])
        nc.gpsimd.dma_start(out=wraw[:, 1, :], in_=w_qkv.ap()[128:256, :])
        woraw = singles.tile([128, 2, 256], F32, tag="woraw")
        nc.gpsimd.dma_start(out=woraw[:, 0, :], in_=w_out.ap()[0:128, :])
        nc.gpsimd.dma_start(out=woraw[:, 1, :], in_=w_out.ap()[128:256, :])
        # rounded (f32r) copies for the PE
        wr = singles.tile([128, 2, 768], F32R, tag="wr")
        nc.vector.tensor_copy(wr, wraw)
        wor = singles.tile([128, 2, 256], F32R, tag="wor")
        nc.vector.tensor_copy(wor, woraw)

        # b_qkv columns: [128, 4] for the 512 q,k biases (per-partition use),
        # [128, 2] (f32r) for the 256 v biases (matmul lhsT use)
        bqk_col = singles.tile([128, 4], F32, tag="bqk_col")
        nc.gpsimd.dma_start(
            out=bqk_col, in_=b_qkv.ap()[0:512].rearrange("(c p) -> p c", p=128)
        )
        bv_raw = singles.tile([128, 2], F32, tag="bv_raw")
        nc.gpsimd.dma_start(
            out=bv_raw, in_=b_qkv.ap()[512:768].rearrange("(c p) -> p c", p=128)
        )
        bv_col = singles.tile([128, 2], F32R, tag="bv_col")
        nc.vector.tensor_copy(bv_col, bv_raw)

        bout_row = singles.tile([1, 256], F32, tag="bout_row")
        nc.gpsimd.dma_start(out=bout_row, in_=b_out.ap()[None, :])

        # ones vectors (f32r via rounding copy; memset bits would likely be
        # fine but the BIR verifier wants f32r-producing ops)
        ones_f32 = singles.tile([1, 128], F32, tag="ones_f32")
        nc.vector.memset(ones_f32, 1.0)
        ones_r = singles.tile([1, 128], F32R, tag="ones_r")
        nc.vector.tensor_copy(ones_r, ones_f32)

        ones_col4 = singles.tile([128, 4], F32, tag="ones_col4")
        nc.vector.memset(ones_col4, 1.0)
        ones_col4r = singles.tile([128, 4], F32R, tag="ones_col4r")
        nc.vector.tensor_copy(ones_col4r, ones_col4)

        # causal 0/1 masks for the 4 diagonal-block offsets:
        # mask[j][p, y] = 1.0 if (y - p - 128*j) >= 0 else 0.0
        masks = []
        for j in range(4):
            mf = singles.tile([128, 512], F32, tag=f"maskf{j}")
            nc.gpsimd.memset(mf, 1.0)
            nc.gpsimd.affine_select(
                out=mf,
                in_=mf,
                compare_op=mybir.AluOpType.is_ge,
                fill=0.0,
                base=-128 * j,
                pattern=[[1, 512]],
                channel_multiplier=-1,
            )
            mr = singles.tile([128, 512], F32R, tag=f"maskr{j}")
            nc.vector.tensor_copy(mr, mf)
            masks.append(mr)

        # combined output bias bc = b_v @ W_out + b_out, broadcast to [128, 256]
        ps_bv = ps.tile([1, 256], F32, tag="ps256")
        for j in range(2):
            nc.tensor.matmul(
                ps_bv, bv_col[:, j : j + 1], wor[:, j, :], start=(j == 0), stop=(j == 1)
            )
        bc_row = singles.tile([1, 256], F32R, tag="bc_row")
        nc.vector.tensor_add(bc_row, ps_bv, bout_row)
        ps_bc = ps.tile([128, 256], F32, tag="ps256")
        nc.tensor.matmul(ps_bc, ones_r, bc_row, start=True, stop=True)
        bc_sb = singles.tile([128, 256], F32, tag="bc_sb")
        nc.scalar.activation(out=bc_sb, in_=ps_bc, func=AF.Identity)

        # ---------------- per-batch pipeline ----------------
        for b in [b for _ in range(reps) for b in range(BPC)]:
            # ---- stage A: load x, build xT [2][128, 1024] (f32r) ----
            xin = []
            for sc in range(SC):
                t = sb.tile([128, 256], F32, tag=f"xin{sc}")
                nc.gpsimd.dma_start(out=t, in_=x.ap()[b, sc * 128 : (sc + 1) * 128, :])
                xin.append(t)
            xT = [sb.tile([128, 1024], F32R, tag=f"xT{ec}", name=f"xT{ec}") for ec in range(2)]
            for ec in range(2):
                for g in range(2):
                    ps_t = ps.tile([128, 512], F32, tag="ps512")
                    for i in range(4):
                        nc.tensor.transpose(
                            ps_t[:, i * 128 : (i + 1) * 128],
                            xin[g * 4 + i][:, ec * 128 : (ec + 1) * 128],
                            ident,
                        )
                    nc.scalar.activation(
                        out=xT[ec][:, g * 512 : (g + 1) * 512], in_=ps_t, func=AF.Identity
                    )

            # ---- stage B: qkT [4][128, 1024] = (W_qk)^T xT + b (f32r) ----
            qkT = [sb.tile([128, 1024], F32R, tag=f"qkT{m}", name=f"qkT{m}") for m in range(4)]
            for m in range(4):
                for qb in range(QN):
                    ps_qk = ps.tile([128, 512], F32, tag="ps512")
                    for ec in range(2):
                        nc.tensor.matmul(
                            ps_qk,
                            wr[:, ec, m * 128 : (m + 1) * 128],
                            xT[ec][:, qb * 512 : (qb + 1) * 512],
                            start=(ec == 0),
                            stop=(ec == 1),
                        )
                    nc.scalar.activation(
                        out=qkT[m][:, qb * 512 : (qb + 1) * 512],
                        in_=ps_qk,
                        func=AF.Identity,
                        bias=bqk_col[:, m : m + 1],
                    )

            # ---- stage B2: V_ext [8][128, 260] natural V + ones cols ----
            vext = []
            for sc in range(SC):
                ps_v = ps.tile([128, 256], F32, tag="ps256")
                for ec in range(2):
                    nc.tensor.matmul(
                        ps_v,
                        xT[ec][:, sc * 128 : (sc + 1) * 128],
                        wr[:, ec, 512:768],
                        start=(ec == 0),
                        stop=(ec == 1),
                    )
                v = sb.tile([128, 4, 65], F32R, tag=f"vext{sc}")
                nc.vector.tensor_copy(
                    v[:, :, 0:64], ps_v.rearrange("p (h d) -> p h d", d=64)
                )
                nc.vector.tensor_copy(v[:, :, 64:65], ones_col4r[:, :, None])
                vext.append(v)

            # ---- stage C: attention per head ----
            ocatT = [sb.tile([128, 1024], F32R, tag=f"ocatT{hc}", name=f"ocatT{hc}") for hc in range(2)]
            for h in range(H):
                hc, hr = h // 2, (h % 2) * 64
                qT = qkT[hc]
                kT = qkT[2 + hc]
                for qb in range(QN):
                    ps_o = ps.tile([65, 512], F32, tag="pso")
                    kmax = 4 * qb + 3
                    for kc in range(kmax + 1):
                        ps_s = ps.tile([128, 512], F32, tag="ps512")
                        nc.tensor.matmul(
                            ps_s,
                            kT[hr : hr + 64, kc * 128 : (kc + 1) * 128],
                            qT[hr : hr + 64, qb * 512 : (qb + 1) * 512],
                            start=True,
                            stop=True,
                        )
                        pT = sb.tile([128, 512], F32R, tag="pT")
                        nc.scalar.activation(out=pT, in_=ps_s, func=AF.Exp, scale=0.125)
                        j = kc - 4 * qb
                        if j >= 0:
                            nc.vector.tensor_mul(pT, pT, masks[j])
                        nc.tensor.matmul(
                            ps_o,
                            vext[kc][:, h, :],
                            pT,
                            start=(kc == 0),
                            stop=(kc == kmax),
                        )
                    # normalize: ocatT rows [hr:hr+64] = psum_o * bcast(1/denom)
                    recip = sb.tile([1, 512], F32R, tag="recip")
                    with nc.allow_low_precision(reason="f32r rounding of softmax denom reciprocals"):
                        nc.vector.reciprocal(recip, ps_o[64:65, :])
                    ps_n = ps.tile([64, 512], F32, tag="psn")
                    nc.tensor.matmul(ps_n, ones_r[:, 0:64], recip, start=True, stop=True)
                    sb_n = sb.tile([64, 512], F32, tag="sbn")
                    nc.vector.tensor_copy(sb_n, ps_n)
                    nc.vector.tensor_mul(
                        ocatT[hc][hr : hr + 64, qb * 512 : (qb + 1) * 512],
                        ps_o[0:64, :],
                        sb_n,
                    )

            # ---- stage D: output projection + combined bias ----
            for sc in range(SC):
                ps_out = ps.tile([128, 256], F32, tag="ps256")
                for hc in range(2):
                    nc.tensor.matmul(
                        ps_out,
                        ocatT[hc][:, sc * 128 : (sc + 1) * 128],
                        wor[:, hc, :],
                        start=(hc == 0),
                        stop=(hc == 1),
                    )
                o = sb.tile([128, 256], F32, tag="osb")
                nc.vector.tensor_add(o, ps_out, bc_sb)
                nc.gpsimd.dma_start(
                    out=out.ap()[b, sc * 128 : (sc + 1) * 128, :], in_=o
                )


# ---------------- host-side runner ----------------
_RUNNER = {}


class _Runner:
    """Compile once, run many: replicates bass2jax.run_bass_via_pjrt's
    shard_map-over-8-devices path but caches the jitted callable."""

    def __init__(self, reps: int = 1):
        import jax
        from jax.sharding import Mesh, PartitionSpec
        from jax.experimental.shard_map import shard_map
        from concourse import bass2jax

        bass2jax.install_neuronx_cc_hook()
        nc = build_nc(reps)
        self.nc = nc

        partition_name = nc.partition_id_tensor.name if nc.partition_id_tensor else None
        in_names, out_names, out_avals, zero_outs = [], [], [], []
        for alloc in nc.m.functions[0].allocations:
            if not isinstance(alloc, mybir.MemoryLocationSet):
                continue
            name = alloc.memorylocations[0].name
            if alloc.kind == "ExternalInput":
                if name != partition_name:
                    in_names.append(name)
            elif alloc.kind == "ExternalOutput":
                out_names.append(name)
                shape = tuple(alloc.tensor_shape)
                dtype = mybir.dt.np(alloc.dtype)
                out_avals.append(jax.core.ShapedArray(shape, dtype))
                zero_outs.append(np.zeros(shape, dtype))
        self.n_params = len(in_names)
        n_outs = len(out_avals)
        self.in_names = list(in_names)
        self.out_names = out_names
        self.out_avals = out_avals
        self.zero_outs = zero_outs
        in_names = in_names + out_names
        if partition_name is not None:
            in_names.append(partition_name)

        def _body(*args):
            operands = list(args)
            if partition_name is not None:
                operands.append(bass2jax.partition_id_tensor())
            outs = bass2jax._bass_exec_p.bind(
                *operands,
                out_avals=tuple(out_avals),
                in_names=tuple(in_names),
                out_names=tuple(out_names),
                lowering_input_output_aliases=(),
                sim_require_finite=True,
                sim_require_nnan=True,
                nc=nc,
            )
            return tuple(outs)

        devices = jax.devices()[:N_CORES]
        mesh = Mesh(np.asarray(devices), ("core",))
        in_specs = (PartitionSpec("core"),) * (self.n_params + n_outs)
        out_specs = (PartitionSpec("core"),) * len(out_names)
        self.fn = jax.jit(
            shard_map(
                _body, mesh=mesh, in_specs=in_specs, out_specs=out_specs, check_rep=False
            ),
            donate_argnums=tuple(range(self.n_params, self.n_params + n_outs)),
            keep_unused=True,
        )

    def concat_inputs(self, in_maps):
        return [
            np.concatenate([np.asarray(m[name]) for m in in_maps], axis=0)
            for name in self.in_names
        ]

    def run_async(self, concat_in):
        concat_zeros = [
            np.zeros((N_CORES * z.shape[0], *z.shape[1:]), z.dtype)
            for z in self.zero_outs
        ]
        return self.fn(*concat_in, *concat_zeros)

    def run(self, in_maps):
        out_arrs = self.run_async(self.concat_inputs(in_maps))
        return [
            {
                name: np.asarray(out_arrs[i]).reshape(
                    N_CORES, *self.out_avals[i].shape
                )[c]
                for i, name in enumerate(self.out_names)
            }
            for c in range(N_CORES)
        ]


def _get_runner(reps: int = 1):
    if reps not in _RUNNER:
        _RUNNER[reps] = _Runner(reps)
    return _RUNNER[reps]


def kernel(x, W_qkv, b_qkv, W_out, b_out):
    x = np.ascontiguousarray(np.asarray(x, dtype=np.float32))
    in_maps = [
        {
            "x": x[c * BPC : (c + 1) * BPC],
            "W_qkv": np.asarray(W_qkv, np.float32),
            "b_qkv": np.asarray(b_qkv, np.float32),
            "W_out": np.asarray(W_out, np.float32),
            "b_out": np.asarray(b_out, np.float32),
        }
        for c in range(N_CORES)
    ]
    results = _get_runner().run(in_maps)
    return np.concatenate([r["out"] for r in results], axis=0)


# revision 7
# speedup vs baseline: 11006.7057x; 157.9583x over previous
"""Causal multi-head attention block (B=16, S=1024, E=256, H=4, D=64) on 8
Trainium2 NeuronCores, data-parallel over batch (2 batches per core).

Layout strategy (per core, per batch):
  - x is PE-transposed to xT [E, S] so QKV projections contract E on the
    partition dim.
  - Q,K are produced TRANSPOSED (qkT [d, S]); V is produced in natural [S, d]
    layout with a ones column appended per head (V_ext) so the P@V matmul
    also yields the softmax denominators (row 64 of the accumulator).
  - Scores are computed as S^T = K Q^T [k, q]; softmax needs no max-subtract
    (scores ~ N(0,1)), so P^T = exp(S^T/8) with causal masking as a 0/1
    multiply on the 4 diagonal block types.
  - Õ^T = V_ext^T P^T accumulates over k-chunks in PSUM; normalization is
    psum * broadcast(1/denom), broadcast done with a K=1 PE matmul.
  - Õ^T is exactly the lhsT the output projection wants.
  - The V-bias is folded through attention algebra into a single combined
    output bias  bc = b_v @ W_out + b_out,  computed once on device.

All matmul operands use float32r (~13-bit mantissa, 4x faster than f32 on
the PE); accumulation stays f32 in PSUM.
"""
import sys

for p in ("/opt/trn_rl_repo",):
    if p not in sys.path:
        sys.path.insert(0, p)

import numpy as np

import concourse.bass as bass
import concourse.mybir as mybir
import concourse.tile as tile
from concourse.masks import make_identity
from concourse.vector_clock import ScopedClock

F32 = mybir.dt.float32
F32R = mybir.dt.float32r
AF = mybir.ActivationFunctionType

N_CORES = 8
B, S, E, H, D = 16, 1024, 256, 4, 64
BPC = B // N_CORES          # batches per core
SC = S // 128               # 128-row s-chunks
QN = S // 512               # 512-col q blocks
WAIT_CAP = 1


class TC(tile.TileContext):
    """TileContext workaround: this walrus build accepts at most one sync
    wait per instruction, so excess waits are peeled onto same-engine NOPs
    emitted immediately before the owning instruction (same semantics: the
    engine blocks on each in order)."""

    def _split_excess_waits(self, inst):
        si = inst.sync_info
        if si is None or len(si.on_wait) <= WAIT_CAP:
            return []
        waits = list(si.on_wait)
        kept, extra = waits[-WAIT_CAP:], waits[:-WAIT_CAP]
        nops = []
        for w in extra:
            nops.append(
                mybir.InstNoOp(
                    name=self.nc.get_next_instruction_name(),
                    engine=inst.engine,
                    sync_info=mybir.SyncInfo(on_wait=[w], on_update=[]),
                    ins=[],
                    outs=[],
                    bass_nofuse=True,
                )
            )
        inst.sync_info = mybir.SyncInfo(on_wait=kept, on_update=list(si.on_update))
        return nops

    def _add_instruction(self, inst):
        for n in self._split_excess_waits(inst):
            super()._add_instruction(n)
        super()._add_instruction(inst)

    def _drain_and_barrier(self, tick_clock, wait_clock):
        probe = self.nc.sync.nop(nofuse=True)
        wait_clock.add_sem_waits(probe.ins, ScopedClock({None: tick_clock.global_clock}))
        si = probe.ins.sync_info
        waits = list(si.on_wait) if si is not None else []
        probe.ins.sync_info = mybir.SyncInfo(on_wait=waits[:1], on_update=[])
        for w in waits[1:]:
            n2 = self.nc.sync.nop(nofuse=True)
            n2.ins.sync_info = mybir.SyncInfo(on_wait=[w], on_update=[])
        self.nc.sync.drain()
        self.nc.all_engine_barrier()
        assert self.sems is not None
        popped = self.nc._tile_sem_poison_stack.pop()
        assert popped is self._sem_poison
        self.nc.clear_and_free_semaphores(list(self.sems.allocated().values()))
        self.nc.all_engine_barrier()


def build_nc(reps: int = 1):
    nc = bass.Bass()
    x = nc.dram_tensor("x", [BPC, S, E], F32, kind="ExternalInput")
    w_qkv = nc.dram_tensor("W_qkv", [E, 3 * E], F32, kind="ExternalInput")
    b_qkv = nc.dram_tensor("b_qkv", [3 * E], F32, kind="ExternalInput")
    w_out = nc.dram_tensor("W_out", [E, E], F32, kind="ExternalInput")
    b_out = nc.dram_tensor("b_out", [E], F32, kind="ExternalInput")
    out = nc.dram_tensor("out", [BPC, S, E], F32, kind="ExternalOutput")

    with TC(nc) as tc:
        _emit(nc, tc, x, w_qkv, b_qkv, w_out, b_out, out, reps)
    return nc


def _emit(nc, tc, x, w_qkv, b_qkv, w_out, b_out, out, reps=1):
    import contextlib

    ctx = contextlib.ExitStack()
    with ctx:
        singles = ctx.enter_context(tc.tile_pool(name="singles", bufs=1))
        sb = ctx.enter_context(tc.tile_pool(name="sb", bufs=2))
        ps = ctx.enter_context(tc.tile_pool(name="ps", bufs=2, space="PSUM"))

        # ---------------- one-time setup ----------------
        ident = singles.tile([128, 128], F32, tag="ident")
        make_identity(nc, ident)

        # raw weight loads
        wraw = singles.tile([128, 2, 768], F32, tag="wraw")
        nc.gpsimd.dma_start(out=wraw[:, 0, :], in_=w_qkv.ap()[0:128, :])
        nc.gpsimd.dma_start(out=wraw[:, 1, :], in_=w_qkv.ap()[128:256, :])
        woraw = singles.tile([128, 2, 256], F32, tag="woraw")
        nc.gpsimd.dma_start(out=woraw[:, 0, :], in_=w_out.ap()[0:128, :])
        nc.gpsimd.dma_start(out=woraw[:, 1, :], in_=w_out.ap()[128:256, :])
        # rounded (f32r) copies for the PE
        wr = singles.tile([128, 2, 768], F32R, tag="wr")
        nc.vector.tensor_copy(wr, wraw)
        wor = singles.tile([128, 2, 256], F32R, tag="wor")
        nc.vector.tensor_copy(wor, woraw)

        # b_qkv columns: [128, 4] for the 512 q,k biases (per-partition use),
        # [128, 2] (f32r) for the 256 v biases (matmul lhsT use)
        bqk_col = singles.tile([128, 4], F32, tag="bqk_col")
        nc.gpsimd.dma_start(
            out=bqk_col, in_=b_qkv.ap()[0:512].rearrange("(c p) -> p c", p=128)
        )
        bv_raw = singles.tile([128, 2], F32, tag="bv_raw")
        nc.gpsimd.dma_start(
            out=bv_raw, in_=b_qkv.ap()[512:768].rearrange("(c p) -> p c", p=128)
        )
        bv_col = singles.tile([128, 2], F32R, tag="bv_col")
        nc.vector.tensor_copy(bv_col, bv_raw)

        bout_row = singles.tile([1, 256], F32, tag="bout_row")
        nc.gpsimd.dma_start(out=bout_row, in_=b_out.ap()[None, :])

        # ones vectors (f32r via rounding copy; memset bits would likely be
        # fine but the BIR verifier wants f32r-producing ops)
        ones_f32 = singles.tile([1, 128], F32, tag="ones_f32")
        nc.vector.memset(ones_f32, 1.0)
        ones_r = singles.tile([1, 128], F32R, tag="ones_r")
        nc.vector.tensor_copy(ones_r, ones_f32)

        # composite causal 0/1 mask for the 4 diagonal-block offsets:
        # cmask[p, j, y] = 1.0 if (y - p - 128*j) >= 0 else 0.0
        cmf = singles.tile([128, 4, 512], F32, tag="cmf")
        nc.gpsimd.memset(cmf, 1.0)
        nc.gpsimd.affine_select(
            out=cmf,
            in_=cmf,
            compare_op=mybir.AluOpType.is_ge,
            fill=0.0,
            base=0,
            pattern=[[-128, 4], [1, 512]],
            channel_multiplier=-1,
        )
        cmask = singles.tile([128, 4, 512], F32R, tag="cmask")
        nc.vector.tensor_copy(cmask, cmf)

        ones32 = singles.tile([128, 8, 4], F32, tag="ones32")
        nc.vector.memset(ones32, 1.0)
        ones32r = singles.tile([128, 8, 4], F32R, tag="ones32r")
        nc.vector.tensor_copy(ones32r, ones32)

        # combined output bias bc = b_v @ W_out + b_out, broadcast to [128, 256]
        ps_bv = ps.tile([1, 256], F32, tag="ps256")
        for j in range(2):
            nc.tensor.matmul(
                ps_bv, bv_col[:, j : j + 1], wor[:, j, :], start=(j == 0), stop=(j == 1)
            )
        bc_row = singles.tile([1, 256], F32R, tag="bc_row")
        nc.vector.tensor_add(bc_row, ps_bv, bout_row)
        ps_bc = ps.tile([128, 256], F32, tag="ps256")
        nc.tensor.matmul(ps_bc, ones_r, bc_row, start=True, stop=True)
        bc_sb = singles.tile([128, 256], F32, tag="bc_sb")
        nc.scalar.activation(out=bc_sb, in_=ps_bc, func=AF.Identity)

        # ---------------- per-batch pipeline ----------------
        for b in [b for _ in range(reps) for b in range(BPC)]:
            # ---- stage A: load x (2 DMAs), build xT [2][128, 1024] (f32r) ----
            xing = []
            for g in range(2):
                t = sb.tile([128, 4, 256], F32, tag=f"xing{g}", name=f"xing{g}")
                nc.gpsimd.dma_start(
                    out=t,
                    in_=x.ap()[b, g * 512 : (g + 1) * 512, :].rearrange(
                        "(j p) e -> p j e", j=4
                    ),
                )
                xing.append(t)
            xT = [sb.tile([128, 1024], F32R, tag=f"xT{ec}", name=f"xT{ec}") for ec in range(2)]
            for ec in range(2):
                for g in range(2):
                    ps_t = ps.tile([128, 512], F32, tag="ps512", bufs=3)
                    for i in range(4):
                        nc.tensor.transpose(
                            ps_t[:, i * 128 : (i + 1) * 128],
                            xing[g][:, i, ec * 128 : (ec + 1) * 128],
                            ident,
                        )
                    nc.scalar.activation(
                        out=xT[ec][:, g * 512 : (g + 1) * 512], in_=ps_t, func=AF.Identity
                    )

            # ---- stage B: qkT [4][128, 1024] = (W_qk)^T xT + b (f32r) ----
            qkT = [sb.tile([128, 1024], F32R, tag=f"qkT{m}", name=f"qkT{m}") for m in range(4)]
            for m in range(4):
                for qb in range(QN):
                    ps_qk = ps.tile([128, 512], F32, tag="ps512", bufs=3)
                    for ec in range(2):
                        nc.tensor.matmul(
                            ps_qk,
                            wr[:, ec, m * 128 : (m + 1) * 128],
                            xT[ec][:, qb * 512 : (qb + 1) * 512],
                            start=(ec == 0),
                            stop=(ec == 1),
                        )
                    nc.scalar.activation(
                        out=qkT[m][:, qb * 512 : (qb + 1) * 512],
                        in_=ps_qk,
                        func=AF.Identity,
                        bias=bqk_col[:, m : m + 1],
                    )

            # ---- stage B2: V_ext [128, 8, 4, 65] natural V + ones cols ----
            vext_all = sb.tile([128, 8, 4, 65], F32R, tag="vext")
            nc.vector.tensor_copy(vext_all[:, :, :, 64:65], ones32r[:, :, :, None])
            for sc2 in range(4):
                ps_v2 = ps.tile([128, 2, 256], F32, tag="ps256")
                for j in range(2):
                    for ec in range(2):
                        nc.tensor.matmul(
                            ps_v2[:, j, :],
                            xT[ec][:, (2 * sc2 + j) * 128 : (2 * sc2 + j + 1) * 128],
                            wr[:, ec, 512:768],
                            start=(ec == 0),
                            stop=(ec == 1),
                        )
                nc.vector.tensor_copy(
                    vext_all[:, 2 * sc2 : 2 * sc2 + 2, :, 0:64],
                    ps_v2.rearrange("p j (h d) -> p j h d", d=64),
                )

            # ---- stage C: attention per head ----
            ocatT = [sb.tile([128, 1024], F32R, tag=f"ocatT{hc}", name=f"ocatT{hc}") for hc in range(2)]
            for h in range(H):
                hc, hr = h // 2, (h % 2) * 64
                qT = qkT[hc]
                kT = qkT[2 + hc]
                for qb in range(QN):
                    ps_o = ps.tile([65, 512], F32, tag="pso", bufs=3)
                    kmax = 4 * qb + 3
                    # full (unmasked) k-blocks: kc in [0, 4*qb)
                    for kc in range(4 * qb):
                        ps_s = ps.tile([128, 512], F32, tag="ps512", bufs=3)
                        nc.tensor.matmul(
                            ps_s,
                            kT[hr : hr + 64, kc * 128 : (kc + 1) * 128],
                            qT[hr : hr + 64, qb * 512 : (qb + 1) * 512],
                            start=True,
                            stop=True,
                        )
                        pT = sb.tile([128, 512], F32R, tag="pT", bufs=4)
                        nc.scalar.activation(out=pT, in_=ps_s, func=AF.Exp, scale=0.125)
                        nc.tensor.matmul(
                            ps_o,
                            vext_all[:, kc, h, :],
                            pT,
                            start=(kc == 0),
                            stop=False,
                        )
                    # diagonal group kc = 4*qb + j, j = 0..3: one composite mask op
                    pTq = sb.tile([128, 4, 512], F32R, tag="pTq")
                    for j in range(4):
                        ps_s = ps.tile([128, 512], F32, tag="ps512", bufs=3)
                        nc.tensor.matmul(
                            ps_s,
                            kT[hr : hr + 64, (4 * qb + j) * 128 : (4 * qb + j + 1) * 128],
                            qT[hr : hr + 64, qb * 512 : (qb + 1) * 512],
                            start=True,
                            stop=True,
                        )
                        nc.scalar.activation(
                            out=pTq[:, j, :], in_=ps_s, func=AF.Exp, scale=0.125
                        )
                    nc.vector.tensor_mul(pTq, pTq, cmask)
                    for j in range(4):
                        kc = 4 * qb + j
                        nc.tensor.matmul(
                            ps_o,
                            vext_all[:, kc, h, :],
                            pTq[:, j, :],
                            start=(kc == 0),
                            stop=(kc == kmax),
                        )
                    # normalize: ocatT rows [hr:hr+64] = psum_o * bcast(1/denom)
                    recip = sb.tile([1, 512], F32R, tag="recip")
                    with nc.allow_low_precision(reason="f32r rounding of softmax denom reciprocals"):
                        nc.vector.reciprocal(recip, ps_o[64:65, :])
                    ps_n = ps.tile([64, 512], F32, tag="pso", bufs=3)
                    nc.tensor.matmul(ps_n, ones_r[:, 0:64], recip, start=True, stop=True)
                    sb_n = sb.tile([64, 512], F32, tag="sbn")
                    nc.vector.tensor_copy(sb_n, ps_n)
                    nc.vector.tensor_mul(
                        ocatT[hc][hr : hr + 64, qb * 512 : (qb + 1) * 512],
                        ps_o[0:64, :],
                        sb_n,
                    )

            # ---- stage D: output projection + combined bias ----
            for sc2 in range(4):
                ps_out2 = ps.tile([128, 2, 256], F32, tag="ps256")
                for j in range(2):
                    for hcc in range(2):
                        nc.tensor.matmul(
                            ps_out2[:, j, :],
                            ocatT[hcc][:, (2 * sc2 + j) * 128 : (2 * sc2 + j + 1) * 128],
                            wor[:, hcc, :],
                            start=(hcc == 0),
                            stop=(hcc == 1),
                        )
                o2 = sb.tile([128, 2, 256], F32, tag="osb")
                for j in range(2):
                    nc.vector.tensor_add(o2[:, j, :], ps_out2[:, j, :], bc_sb)
                nc.gpsimd.dma_start(
                    out=out.ap()[b, sc2 * 256 : (sc2 + 1) * 256, :].rearrange(
                        "(j p) e -> p j e", j=2
                    ),
                    in_=o2,
                )


# ---------------- host-side runner ----------------
_RUNNER = {}


class _Runner:
    """Compile once, run many: replicates bass2jax.run_bass_via_pjrt's
    shard_map-over-8-devices path but caches the jitted callable."""

    def __init__(self, reps: int = 1):
        import jax
        from jax.sharding import Mesh, PartitionSpec
        from jax.experimental.shard_map import shard_map
        from concourse import bass2jax

        bass2jax.install_neuronx_cc_hook()
        nc = build_nc(reps)
        self.nc = nc

        partition_name = nc.partition_id_tensor.name if nc.partition_id_tensor else None
        in_names, out_names, out_avals, zero_outs = [], [], [], []
        for alloc in nc.m.functions[0].allocations:
            if not isinstance(alloc, mybir.MemoryLocationSet):
                continue
            name = alloc.memorylocations[0].name
            if alloc.kind == "ExternalInput":
                if name != partition_name:
                    in_names.append(name)
            elif alloc.kind == "ExternalOutput":
                out_names.append(name)
                shape = tuple(alloc.tensor_shape)
                dtype = mybir.dt.np(alloc.dtype)
                out_avals.append(jax.core.ShapedArray(shape, dtype))
                zero_outs.append(np.zeros(shape, dtype))
        self.n_params = len(in_names)
        n_outs = len(out_avals)
        self.in_names = list(in_names)
        self.out_names = out_names
        self.out_avals = out_avals
        self.zero_outs = zero_outs
        in_names = in_names + out_names
        if partition_name is not None:
            in_names.append(partition_name)

        def _body(*args):
            operands = list(args)
            if partition_name is not None:
                operands.append(bass2jax.partition_id_tensor())
            outs = bass2jax._bass_exec_p.bind(
                *operands,
                out_avals=tuple(out_avals),
                in_names=tuple(in_names),
                out_names=tuple(out_names),
                lowering_input_output_aliases=(),
                sim_require_finite=True,
                sim_require_nnan=True,
                nc=nc,
            )
            return tuple(outs)

        devices = jax.devices()[:N_CORES]
        mesh = Mesh(np.asarray(devices), ("core",))
        in_specs = (PartitionSpec("core"),) * (self.n_params + n_outs)
        out_specs = (PartitionSpec("core"),) * len(out_names)
        self.fn = jax.jit(
            shard_map(
                _body, mesh=mesh, in_specs=in_specs, out_specs=out_specs, check_rep=False
            ),
            donate_argnums=tuple(range(self.n_params, self.n_params + n_outs)),
            keep_unused=True,
        )

    def concat_inputs(self, in_maps):
        return [
            np.concatenate([np.asarray(m[name]) for m in in_maps], axis=0)
            for name in self.in_names
        ]

    def run_async(self, concat_in):
        concat_zeros = [
            np.zeros((N_CORES * z.shape[0], *z.shape[1:]), z.dtype)
            for z in self.zero_outs
        ]
        return self.fn(*concat_in, *concat_zeros)

    def run(self, in_maps):
        out_arrs = self.run_async(self.concat_inputs(in_maps))
        return [
            {
                name: np.asarray(out_arrs[i]).reshape(
                    N_CORES, *self.out_avals[i].shape
                )[c]
                for i, name in enumerate(self.out_names)
            }
            for c in range(N_CORES)
        ]


def _get_runner(reps: int = 1):
    if reps not in _RUNNER:
        _RUNNER[reps] = _Runner(reps)
    return _RUNNER[reps]


def kernel(x, W_qkv, b_qkv, W_out, b_out):
    x = np.ascontiguousarray(np.asarray(x, dtype=np.float32))
    in_maps = [
        {
            "x": x[c * BPC : (c + 1) * BPC],
            "W_qkv": np.asarray(W_qkv, np.float32),
            "b_qkv": np.asarray(b_qkv, np.float32),
            "W_out": np.asarray(W_out, np.float32),
            "b_out": np.asarray(b_out, np.float32),
        }
        for c in range(N_CORES)
    ]
    results = _get_runner().run(in_maps)
    return np.concatenate([r["out"] for r in results], axis=0)
